# revision 43
# baseline (speedup 1.0000x reference)
"""AttnBlock (GroupNorm -> QKV 1x1 -> full NxN attention -> proj -> residual)
for Trainium2, SPMD over 8 NeuronCores.

Sharding: data-parallel over batch (2) x query-pixel blocks (4 of 1024 px).
Each core receives its batch image x [C, N]; K and V^T are computed
redundantly per batch group, queries are disjoint.  The host ROTATES the
pixel axis per core so each core's query block is always columns [0, NQ)
of its x copy (attention is permutation-invariant over keys, GN stats are
order-free), so no separate xq input is needed and the residual is read
from resident x (bf16: ~1.5e-3 rel err vs the 2e-2 gate).  No collectives.

ALL inputs are packed into ONE bf16 DRAM tensor (fp32 sections stored as
byte pairs and bitcast at use): the axon PJRT proxy charges ~40us of
dispatch overhead PER ARGUMENT per call, so the original 17 inputs cost
~0.7ms of pure dispatch per invocation.

Per-core structure (precision="bf16", the default):
  0. DMA everything once on ONE prioritized queue (x pieces first, 4KB
     contiguous per partition); x stays RESIDENT in SBUF as bf16.
  1. GroupNorm stats split across engines so neither serializes the head:
     DVE bn_stats on 6 windows per channel-subtile, ACT raw sum/sum-sq
     (accum_out) on 2; merged, then tiny indicator matmuls do the
     cross-partition group reduce -> per-channel A, B
  2. fused single sweep over resident x chunks: hn = A*x + B (bf16) ->
     matmuls into resident K [c,n] and V^T [n,c]; Q emitted for the first
     NQ/512 chunks (attn scale folded into wq/bq on host; k-bias dropped:
     softmax over keys is invariant to a per-query additive constant)
  3. attention per 512-query pass, streamed over 32 key tiles:
     S[k,q] = K^T@Q -> exp (no max subtraction: scores ~ N(0,1)) -> pt
     bf16; AV accumulated in PSUM directly in [c,q] layout (lhsT = V^T
     tile) so no PE transposes are needed; softmax denominators: DVE
     running acc over pt tiles, then ONE all-ones matmul broadcasts
     per-query sums to all partitions; normalization is folded AFTER
     proj: out = (Wp O) * rec + (bp + Wp bv folded) + x, computed in
     half-tiles pipelined across DVE -> DMA so the final drain is short.

Matmul operands are bf16 (1 PE cycle/row, vs 4 for fp32); PSUM
accumulation is fp32.  Cost-model device time ~214us vs ~785us for the
original fp32 kernel (PE busy ~179us of that).
precision="tf32"/"fp32" build the v1 unfused single-pack variant.
"""

from contextlib import ExitStack

import numpy as np

import concourse.bacc as bacc
import concourse.bass as bass
import concourse.mybir as mybir
import concourse.tile as tile

F32 = mybir.dt.float32
F32R = mybir.dt.float32r
BF16 = mybir.dt.bfloat16
AF = mybir.ActivationFunctionType
ALU = mybir.AluOpType


# ---------------------------------------------------------------------------
# v2: bf16 fused kernel
# ---------------------------------------------------------------------------

def pack_offsets_v2(C=512, N=4096, NQ=1024):
    """Column offsets in the single packed [C, Mb] bf16 input tensor.

    fp32 sections are stored as raw bytes occupying 2x bf16 columns and
    bitcast to f32 on the DRAM AP at use (offsets stay even => 4B aligned).
    """
    P = 128
    CS = C // P
    offb, o = {}, 0
    for name, w in (("x", N),
                    ("indg", 2 * 8), ("inde", 2 * P), ("gammaT", 2 * CS),
                    ("betaT", 2 * CS), ("bqT", 2 * CS), ("bpT", 2 * CS),
                    ("wqt", C), ("wkt", C), ("wvt", C), ("wpt", C)):
        offb[name] = (o, w)
        o += w
    return offb, o


def build_program_v2(C=512, G=32, N=4096, NQ=1024, eps=1e-5):
    P = 128
    CS = C // P                  # channel subtiles
    KT = N // P                  # key tiles
    NCH = 512                    # x chunk width (= bn_stats window max)
    NCHUNKS = N // NCH
    QP = 512                     # query-pass width (one PSUM bank of fp32)
    QPASSES = NQ // QP
    cpg = C // G                 # channels per group
    GPS = P // cpg               # groups per channel-subtile
    assert C % P == 0 and N % NCH == 0 and NQ % QP == 0 and P % cpg == 0

    offb, Mb = pack_offsets_v2(C, N, NQ)

    nc = bacc.Bacc(None, target_bir_lowering=False)

    packb_d = nc.dram_tensor("packb", [C, Mb], BF16, kind="ExternalInput")
    out_d = nc.dram_tensor("out", [C, NQ], F32, kind="ExternalOutput")

    def bcol(name):
        o, w = offb[name]
        return packb_d[:, o:o + w]

    def fcol(name, rows=C):
        o, w = offb[name]
        return packb_d[0:rows, o:o + w].bitcast(F32)

    out_r = out_d[:, :].rearrange("(s p) n -> p s n", p=P)

    with tile.TileContext(nc) as tc, ExitStack() as st:
        const = st.enter_context(tc.tile_pool(name="const", bufs=1))
        big = st.enter_context(tc.tile_pool(name="big", bufs=1))
        small = st.enter_context(tc.tile_pool(name="small", bufs=1))
        ps_sh = st.enter_context(tc.tile_pool(name="ps_sh", bufs=3, space="PSUM"))
        ps_o = st.enter_context(tc.tile_pool(name="ps_o", bufs=4, space="PSUM"))
        ps_den = st.enter_context(tc.tile_pool(name="ps_den", bufs=1, space="PSUM"))

        # ---- resident tensors / constants ---------------------------------
        # x[c, n] resident as separate per-(subtile, half) tiles so the
        # phase-1 bn_stats only depend on the piece they read (whole-tile
        # deps would stall stats until the LAST x DMA).
        NH = N // 2
        X_pc = [[big.tile([P, NH], BF16, tag=f"X{s}{hh}", name=f"X_{s}_{hh}")
                 for hh in range(2)] for s in range(CS)]

        def xsl(s, lo, wdt):
            hh, off = divmod(lo, NH)
            assert off + wdt <= NH
            return X_pc[s][hh][:, off:off + wdt]
        K_sb = big.tile([P, CS, N], BF16, tag="K")        # K[co, n]
        VT_sb = big.tile([P, KT, C], BF16, tag="VT")      # V^T[n, co]
        Q_sb = big.tile([P, CS, NQ], BF16, tag="Q")       # Q[co, nq] scaled
        W_sb = {}
        for w in ("q", "k", "v", "p"):
            W_sb[w] = big.tile([P, CS, C], BF16, tag=f"w{w}", name=f"w_{w}")

        # ALL input DMAs go on the sync queue in priority order: x pieces
        # (phase-1 critical path) -> tiny consts -> weights -> xq -> xqf.
        # A single ordered queue stops the later transfers from stealing
        # HBM bandwidth from x, which gates the GroupNorm stats -> A/B ->
        # everything else dependency chain.  x is loaded per (channel
        # subtile, half): [128, 2048] bf16 pieces = 4KB contiguous per
        # partition (efficient DMA), with bn_stats chasing each piece.
        x_r = bcol("x").rearrange("(s p) n -> p s n", p=P)
        for s in range(CS):
            for hh in range(2):
                nc.sync.dma_start(
                    out=X_pc[s][hh],
                    in_=x_r[:, s, hh * NH:(hh + 1) * NH])

        indg = const.tile([P, GPS], F32, tag="indg")
        nc.sync.dma_start(out=indg, in_=fcol("indg", P)[:, 0:GPS])
        inde = const.tile([GPS, P], F32, tag="inde")
        nc.sync.dma_start(out=inde, in_=fcol("inde", GPS))
        gammaT = const.tile([P, CS], F32, tag="gammaT")
        nc.sync.dma_start(out=gammaT, in_=fcol("gammaT", P))
        betaT = const.tile([P, CS], F32, tag="betaT")
        nc.sync.dma_start(out=betaT, in_=fcol("betaT", P))
        bqT = const.tile([P, CS], F32, tag="bqT")
        nc.sync.dma_start(out=bqT, in_=fcol("bqT", P))
        bpT = const.tile([P, CS], F32, tag="bpT")
        nc.sync.dma_start(out=bpT, in_=fcol("bpT", P))

        for w in ("k", "v", "q", "p"):
            nc.sync.dma_start(out=W_sb[w], in_=bcol(f"w{w}t").rearrange(
                "(s p) c -> p s c", p=P))
        onesM_f = const.tile([P, P], F32, tag="onesM_f")
        nc.vector.memset(onesM_f, 1.0)
        onesM = const.tile([P, P], F32R, tag="onesM")
        nc.vector.tensor_copy(out=onesM, in_=onesM_f)
        eps_t = const.tile([P, 1], F32, tag="eps")
        nc.vector.memset(eps_t, eps)

        # ---- phase 1: group-norm stats over resident x --------------------
        # The 32 [128, 512] stat windows are split across two engines so
        # neither serializes the head: per channel-subtile, 6 windows go to
        # DVE bn_stats and 2 (w == 3 of each half) to ACT as raw sum +
        # sum-of-squares via accum_out.  (Pool cannot run TensorScalarPtr,
        # so only ACT can help here.)
        stats_all = small.tile([P, CS, 6, 6], F32, tag="stats")
        sxa = small.tile([P, CS, 2, 2], F32, tag="sxa")  # raw windows
        junk_a = small.tile([P, NCH], F32, tag="junk_a")
        for s in range(CS):
            for hh in range(2):
                for w in range(4):
                    ch = 4 * hh + w
                    xs = xsl(s, ch * NCH, NCH)
                    if w < 3:
                        nc.vector.bn_stats(
                            out=stats_all[:, s, 3 * hh + w, :], in_=xs)
                    else:
                        nc.scalar.activation(
                            out=junk_a, in_=xs, func=AF.Copy,
                            accum_out=sxa[:, s, hh, 0:1])
                        nc.scalar.activation(
                            out=junk_a, in_=xs, func=AF.Square,
                            accum_out=sxa[:, s, hh, 1:2])
        mv = small.tile([P, CS, 2], F32, tag="mv")
        for s in range(CS):
            nc.vector.bn_aggr(out=mv[:, s, :], in_=stats_all[:, s, :, :])

        # merge the DVE windows (mean/var over N_D px per channel) with the
        # ACT raw sums (N - N_D px) into per-channel mean / E[x^2] (rhs8),
        # group-reduced via indicator matmul below.
        N_D = 6.0 * NCH
        sxt = small.tile([P, CS, 2], F32, tag="sxt")
        nc.vector.tensor_add(out=sxt, in0=sxa[:, :, 0, :], in1=sxa[:, :, 1, :])
        rhs8 = small.tile([P, 2 * CS], F32, tag="rhs8")
        # mean_tot = (mean_d * N_D + sx_raw) / N
        nc.vector.tensor_scalar_mul(rhs8[:, 0:CS], mv[:, :, 0], N_D / float(N))
        nc.vector.tensor_scalar_mul(rhs8[:, CS:], sxt[:, :, 0], 1.0 / float(N))
        nc.vector.tensor_add(out=rhs8[:, 0:CS], in0=rhs8[:, 0:CS],
                             in1=rhs8[:, CS:])
        # E[x^2]_tot = ((var_d + mean_d^2) * N_D + sxx_raw) / N
        ex2 = small.tile([P, CS], F32, tag="ex2")
        nc.vector.tensor_mul(out=ex2, in0=mv[:, :, 0], in1=mv[:, :, 0])
        nc.vector.tensor_add(out=ex2, in0=ex2, in1=mv[:, :, 1])
        nc.vector.tensor_scalar_mul(ex2, ex2, N_D / float(N))
        nc.vector.tensor_scalar_mul(rhs8[:, CS:], sxt[:, :, 1], 1.0 / float(N))
        nc.vector.tensor_add(out=rhs8[:, CS:], in0=rhs8[:, CS:], in1=ex2)
        ps_g = ps_sh.tile([GPS, 2 * CS], F32, tag="sbank")
        nc.tensor.matmul(ps_g, lhsT=indg, rhs=rhs8, start=True, stop=True)
        gtmp = small.tile([GPS, 2 * CS], F32, tag="gtmp")
        nc.vector.tensor_scalar_mul(gtmp, ps_g, 1.0 / cpg)
        gsq = small.tile([GPS, CS], F32, tag="gsq")
        nc.vector.tensor_mul(out=gsq, in0=gtmp[:, 0:CS], in1=gtmp[:, 0:CS])
        e8 = small.tile([GPS, 2 * CS], F32, tag="e8")
        nc.vector.tensor_sub(out=e8[:, 0:CS], in0=gtmp[:, CS:], in1=gsq)
        nc.scalar.activation(out=e8[:, 0:CS], in_=e8[:, 0:CS], func=AF.Sqrt,
                             bias=eps_t[:GPS], scale=1.0)
        nc.vector.reciprocal(out=e8[:, 0:CS], in_=e8[:, 0:CS])
        nc.vector.tensor_copy(out=e8[:, CS:], in_=gtmp[:, 0:CS])
        ps_e = ps_sh.tile([P, 2 * CS], F32, tag="sbank")
        nc.tensor.matmul(ps_e, lhsT=inde, rhs=e8, start=True, stop=True)
        A_sb = small.tile([P, CS], F32, tag="A")
        nc.vector.tensor_mul(out=A_sb, in0=ps_e[:, 0:CS], in1=gammaT)
        B_sb = small.tile([P, CS], F32, tag="B")
        nc.vector.tensor_mul(out=B_sb, in0=ps_e[:, CS:], in1=A_sb)
        nc.vector.tensor_sub(out=B_sb, in0=betaT, in1=B_sb)

        # ---- phase 2: fused hn -> K, V^T sweep; then Q --------------------
        with ExitStack() as st1:
            hnp = st1.enter_context(tc.tile_pool(name="hnp", bufs=2))

            def hn_chunk(get, name):
                # one tile per channel-subtile: the first K matmul of the
                # chunk can start as soon as hn[0] lands (finer deps)
                hn = [hnp.tile([P, NCH], BF16, tag=f"hn{s}",
                               name=f"{name}_{s}") for s in range(CS)]
                for s in range(CS):
                    nc.vector.tensor_scalar(
                        hn[s], get(s),
                        scalar1=A_sb[:, s:s + 1], scalar2=B_sb[:, s:s + 1],
                        op0=ALU.mult, op1=ALU.add,
                    )
                return hn

            for ch in range(NCHUNKS):
                hn = hn_chunk(lambda s, c=ch: xsl(s, c * NCH, NCH),
                              f"hn_{ch}")
                for cs in range(CS):          # K rows [co-sub, chunk]
                    ps_k = ps_sh.tile([P, NCH], F32, tag="sbank")
                    for s in range(CS):
                        nc.tensor.matmul(
                            ps_k, lhsT=W_sb["k"][:, s, cs * P:(cs + 1) * P],
                            rhs=hn[s],
                            start=(s == 0), stop=(s == CS - 1),
                        )
                    nc.scalar.activation(
                        out=K_sb[:, cs, ch * NCH:(ch + 1) * NCH], in_=ps_k,
                        func=AF.Copy)
                for ns in range(NCH // P):    # V^T rows [pixel-sub, all co]
                    ps_v = ps_sh.tile([P, C], F32, tag="sbank")
                    for s in range(CS):
                        nc.tensor.matmul(
                            ps_v, lhsT=hn[s][:, ns * P:(ns + 1) * P],
                            rhs=W_sb["v"][:, s, :],
                            start=(s == 0), stop=(s == CS - 1),
                        )
                    nc.vector.tensor_copy(
                        out=VT_sb[:, ch * (NCH // P) + ns, :], in_=ps_v)
                if ch < NQ // NCH:
                    # this core's query block is pixels [0, NQ) (the host
                    # rotates the pixel axis per core), so Q comes from the
                    # same hn chunks as K/V
                    for cs in range(CS):
                        ps_q = ps_sh.tile([P, NCH], F32, tag="sbank")
                        for s in range(CS):
                            nc.tensor.matmul(
                                ps_q,
                                lhsT=W_sb["q"][:, s, cs * P:(cs + 1) * P],
                                rhs=hn[s],
                                start=(s == 0), stop=(s == CS - 1),
                            )
                        nc.scalar.activation(
                            out=Q_sb[:, cs, ch * NCH:(ch + 1) * NCH],
                            in_=ps_q, func=AF.Identity,
                            bias=bqT[:, cs:cs + 1], scale=1.0)

        # ---- phase 3: attention + proj + residual, per query pass ---------
        with ExitStack() as st2:
            ptp = st2.enter_context(tc.tile_pool(name="ptp", bufs=3))
            ocq = st2.enter_context(tc.tile_pool(name="ocq", bufs=1))
            outp = st2.enter_context(tc.tile_pool(name="outp", bufs=2))
            sm2 = st2.enter_context(tc.tile_pool(name="sm2", bufs=2))

            for qp in range(QPASSES):
                q0 = qp * QP
                o_ps = [ps_o.tile([P, QP], F32, tag="o", name=f"o_{qp}_{cs}")
                        for cs in range(CS)]
                # denominator accumulator (f32r so the all-ones matmul can
                # read it; DVE reads go through a f32 bitcast)
                acc = sm2.tile([P, QP], F32R, tag="acc")
                pt_q = []

                def emit_s(kt):
                    s_ps = ps_sh.tile([P, QP], F32, tag="sbank",
                                      name=f"s_ps_{qp}_{kt}")
                    for s in range(CS):
                        nc.tensor.matmul(
                            s_ps, lhsT=K_sb[:, s, kt * P:(kt + 1) * P],
                            rhs=Q_sb[:, s, q0:q0 + QP],
                            start=(s == 0), stop=(s == CS - 1),
                        )
                    pt = ptp.tile([P, QP], BF16, tag="pt",
                                  name=f"pt_{qp}_{kt}")
                    nc.scalar.activation(out=pt, in_=s_ps, func=AF.Exp)
                    pt_q.append((kt, pt))

                emit_s(0)
                for kt in range(KT):
                    if kt + 1 < KT:
                        emit_s(kt + 1)
                    k0, pt = pt_q.pop(0)
                    assert k0 == kt
                    if kt == 0:
                        nc.vector.tensor_copy(out=acc, in_=pt)
                    else:
                        nc.vector.tensor_add(out=acc, in0=acc.bitcast(F32),
                                             in1=pt)
                    last = kt == KT - 1
                    for cs in range(CS):      # O[c, q] directly
                        nc.tensor.matmul(
                            o_ps[cs], lhsT=VT_sb[:, kt, cs * P:(cs + 1) * P],
                            rhs=pt,
                            start=(kt == 0), stop=last,
                        )
                # denominators broadcast to every partition in one matmul
                den_ps = ps_den.tile([P, QP], F32, tag="den")
                nc.tensor.matmul(den_ps, lhsT=onesM, rhs=acc,
                                 start=True, stop=True)
                rec = sm2.tile([P, QP], F32, tag="rec")
                nc.vector.reciprocal(out=rec, in_=den_ps)

                oc = ocq.tile([P, CS, QP], BF16, tag="ocq")
                for cs in range(CS):
                    nc.scalar.activation(out=oc[:, cs, :], in_=o_ps[cs],
                                         func=AF.Copy)
                for cs in range(CS):          # proj rows [co-sub, qpass]
                    ps_p = ps_sh.tile([P, QP], F32, tag="sbank")
                    for s in range(CS):
                        nc.tensor.matmul(
                            ps_p, lhsT=W_sb["p"][:, s, cs * P:(cs + 1) * P],
                            rhs=oc[:, s, :],
                            start=(s == 0), stop=(s == CS - 1),
                        )
                    # epilogue in half-tiles: DVE (psum*rec) -> Pool (+bias
                    # +residual) -> DMA pipeline so the final drain is short
                    HQ = QP // 2
                    for h in range(2):
                        lo = h * HQ
                        t1 = outp.tile([P, HQ], F32, tag=f"t1{h}",
                                       name=f"t1_{qp}_{cs}_{h}")
                        nc.vector.tensor_mul(out=t1, in0=ps_p[:, lo:lo + HQ],
                                             in1=rec[:, lo:lo + HQ])
                        ot = outp.tile([P, HQ], F32, tag=f"ot{h}",
                                       name=f"ot_{qp}_{cs}_{h}")
                        nc.vector.scalar_tensor_tensor(
                            out=ot, in0=t1, scalar=bpT[:, cs:cs + 1],
                            in1=xsl(cs, q0 + lo, HQ),
                            op0=ALU.add, op1=ALU.add)
                        (nc.sync if (2 * cs + h) % 2 == 0 else
                         nc.scalar).dma_start(
                            out=out_r[:, cs, q0 + lo:q0 + lo + HQ], in_=ot)

    nc.finalize()
    return nc


def make_in_maps_v2(x, gn_w, gn_b, q_w, q_b, k_w, k_b, v_w, v_b, proj_w,
                    proj_b, n_cores=8, G=32):
    """Shard full inputs into per-core packed input maps (biases folded)."""
    NPBF = mybir.dt.np(BF16)
    f = lambda a: np.ascontiguousarray(np.asarray(a, dtype=np.float32))
    x = f(x)
    b, c, h, w = x.shape
    n = h * w
    qblocks = n_cores // b
    nq = n // qblocks
    cs = c // 128
    scale = np.float32(c ** -0.5)
    xf = x.reshape(b, c, n)
    offb, Mb = pack_offsets_v2(c, n, nq)

    def to_pcs(v):                       # [C] -> [128, CS] (c = 128*s + p)
        return np.ascontiguousarray(np.asarray(v, np.float32).reshape(cs, 128).T)

    P = 128
    cpg = c // G
    GPS = P // cpg
    indg = np.zeros((P, 8), np.float32)
    for p in range(P):
        indg[p, p // cpg] = 1.0
    inde = np.ascontiguousarray(indg[:, :GPS].T)

    commonb = np.zeros((c, Mb), NPBF)

    def putb(name, arr):
        o, wdt = offb[name]
        commonb[:, o:o + wdt] = np.asarray(arr).astype(NPBF)

    def putf(buf, name, arr):
        # embed raw fp32 bytes into the bf16 pack (2 bf16 cols per f32 col)
        o, wdt = offb[name]
        arr = np.asarray(arr, np.float32)
        rows = arr.shape[0]
        tmp = np.zeros((rows, wdt), NPBF)
        tmp.view(np.float32)[...] = arr
        buf[:rows, o:o + wdt] = tmp

    putb("wqt", f(q_w).T * scale)
    putb("wkt", f(k_w).T)
    putb("wvt", f(v_w).T)
    putb("wpt", f(proj_w).T)
    putf(commonb, "bqT", to_pcs(f(q_b) * scale))
    putf(commonb, "bpT", to_pcs(f(proj_w) @ f(v_b) + f(proj_b)))
    putf(commonb, "gammaT", to_pcs(gn_w))
    putf(commonb, "betaT", to_pcs(gn_b))
    putf(commonb, "indg", indg)
    putf(commonb, "inde", inde)

    in_maps = []
    for i in range(n_cores):
        bi, qi = divmod(i, qblocks)
        pkb = commonb.copy()
        xo, _ = offb["x"]
        # rotate the pixel axis so this core's query block sits at columns
        # [0, nq): attention is permutation-invariant over keys and the GN
        # stats are order-free, so only the query slice selection changes
        pkb[:, xo:xo + n] = np.roll(
            xf[bi], -qi * nq, axis=1).astype(NPBF)
        in_maps.append({"packb": pkb})
    return in_maps, (b, c, h, w, n, nq, qblocks)


# ---------------------------------------------------------------------------
# v1: single-pack fp32/tf32 kernel (kept for comparison; see git history of
# the docstring for the full description)
# ---------------------------------------------------------------------------

def pack_offsets(C=512, N=4096, NQ=1024):
    """Column offsets in the packed [C, M] fp32 input tensor."""
    P = 128
    CS = C // P
    off = {}
    o = 0
    for name, w in (("x", N), ("xq", NQ), ("wqt", C), ("wkt", C),
                    ("wvt", C), ("wpt", C), ("bqT", CS), ("bkT", CS),
                    ("bpT", CS), ("gammaT", CS), ("betaT", CS),
                    ("indg", P // (C // 32)), ("inde", P), ("ident", P)):
        off[name] = (o, w)
        o += w
    return off, o


def build_program(C=512, G=32, N=4096, NQ=1024, eps=1e-5, precision="tf32"):
    """Emit the per-core Bass program (SPMD; per-core data differs only)."""
    P = 128
    CS = C // P                  # channel subtiles
    KT = N // P                  # key/pixel tiles
    NCH = min(512, N)            # streamed x chunk (pixels); also bn window
    NCHUNKS = N // NCH
    QP = min(512, NQ)            # query-pass width
    QPASSES = NQ // QP
    QS = QP // P                 # query subtiles per pass
    cpg = C // G                 # channels per group
    GPS = P // cpg               # groups per channel-subtile
    assert C % P == 0 and N % P == 0 and NQ % QP == 0 and P % cpg == 0
    MMDT = F32R if precision == "tf32" else F32

    off, M = pack_offsets(C, N, NQ)

    nc = bacc.Bacc(None, target_bir_lowering=False)

    pack_d = nc.dram_tensor("pack", [C, M], F32, kind="ExternalInput")
    out_d = nc.dram_tensor("out", [C, NQ], F32, kind="ExternalOutput")

    def pcol(name):
        o, w = off[name]
        return pack_d[:, o:o + w]

    def prows(name, rows):
        o, w = off[name]
        return pack_d[0:rows, o:o + w]

    x_r = pcol("x").rearrange("(s p) n -> p s n", p=P)
    xq_r = pcol("xq").rearrange("(s p) n -> p s n", p=P)
    out_r = out_d[:, :].rearrange("(s p) n -> p s n", p=P)

    with tile.TileContext(nc) as tc, ExitStack() as st:
        const = st.enter_context(tc.tile_pool(name="const", bufs=1))
        big = st.enter_context(tc.tile_pool(name="big", bufs=1))
        small = st.enter_context(tc.tile_pool(name="small", bufs=1))
        ps_sh = st.enter_context(tc.tile_pool(name="ps_sh", bufs=3, space="PSUM"))
        ps_o = st.enter_context(tc.tile_pool(name="ps_o", bufs=QS, space="PSUM"))
        ps_sum = st.enter_context(tc.tile_pool(name="ps_sum", bufs=1, space="PSUM"))

        # ---- constants / params -------------------------------------------
        indg = const.tile([P, GPS], F32, tag="indg")
        nc.sync.dma_start(out=indg, in_=prows("indg", P))
        inde = const.tile([GPS, P], F32, tag="inde")
        nc.sync.dma_start(out=inde, in_=prows("inde", GPS))
        ident = const.tile([P, P], F32, tag="ident")
        nc.sync.dma_start(out=ident, in_=prows("ident", P))
        gammaT = const.tile([P, CS], F32, tag="gammaT")
        nc.sync.dma_start(out=gammaT, in_=prows("gammaT", P))
        betaT = const.tile([P, CS], F32, tag="betaT")
        nc.sync.dma_start(out=betaT, in_=prows("betaT", P))
        bT = {}
        for name in ("q", "k", "p"):
            t = const.tile([P, CS], F32, tag=f"bT_{name}")
            nc.sync.dma_start(out=t, in_=prows(f"b{name}T", P))
            bT[name] = t
        ones_r = const.tile([P, 1], F32, tag="ones_r")
        nc.vector.memset(ones_r, 1.0)
        eps_t = const.tile([P, 1], F32, tag="eps")
        nc.vector.memset(eps_t, eps)

        K_sb = big.tile([P, CS, N], MMDT, tag="K")       # K[co, n]
        VT_sb = big.tile([P, KT, C], MMDT, tag="VT")     # V^T[n, co]
        Q_sb = big.tile([P, CS, NQ], MMDT, tag="Q")      # Q[co, nq] (scaled)
        wpT = big.tile([P, CS, C], MMDT, tag="wpT")      # proj weight

        # ---- phase 1: group-norm stats over streamed x --------------------
        with ExitStack() as st1:
            xch = st1.enter_context(tc.tile_pool(name="xch", bufs=2))
            hnp = st1.enter_context(tc.tile_pool(name="hnp", bufs=2))
            wqkv = st1.enter_context(tc.tile_pool(name="wqkv", bufs=2))

            def load_weight(w, pool, tag):
                if pool is None:
                    t = wpT
                else:
                    t = pool.tile([P, CS, C], MMDT, tag=tag, name=f"w_{w}")
                src = pcol(f"w{w}t").rearrange("(s p) c -> p s c", p=P)
                if MMDT is F32:
                    nc.sync.dma_start(out=t, in_=src)
                else:
                    raw = xch.tile([P, CS, C], F32, tag="xc", name=f"wraw_{w}")
                    nc.sync.dma_start(out=raw, in_=src)
                    nc.vector.tensor_copy(out=t, in_=raw)  # rounds to f32r
                return t

            stats_all = small.tile([P, CS, NCHUNKS, 6], F32, tag="stats")
            dma_engs = [nc.sync, nc.scalar, nc.gpsimd]
            for ch in range(NCHUNKS):
                xc = xch.tile([P, CS, NCH], F32, tag="xc")
                dma_engs[ch % len(dma_engs)].dma_start(
                    out=xc, in_=x_r[:, :, ch * NCH:(ch + 1) * NCH])
                for s in range(CS):
                    nc.vector.bn_stats(out=stats_all[:, s, ch, :], in_=xc[:, s, :])
            mv = small.tile([P, CS, 2], F32, tag="mv")
            for s in range(CS):
                nc.vector.bn_aggr(out=mv[:, s, :], in_=stats_all[:, s, :, :])

            rhs8 = small.tile([P, 2 * CS], F32, tag="rhs8")
            nc.vector.tensor_copy(out=rhs8[:, 0:CS], in_=mv[:, :, 0])
            nc.vector.tensor_mul(out=rhs8[:, CS:], in0=mv[:, :, 0], in1=mv[:, :, 0])
            nc.vector.tensor_add(out=rhs8[:, CS:], in0=rhs8[:, CS:], in1=mv[:, :, 1])
            ps_g = ps_sh.tile([GPS, 2 * CS], F32, tag="sbank")
            nc.tensor.matmul(ps_g, lhsT=indg, rhs=rhs8, start=True, stop=True)
            gtmp = small.tile([GPS, 2 * CS], F32, tag="gtmp")
            nc.vector.tensor_scalar_mul(gtmp, ps_g, 1.0 / cpg)
            gsq = small.tile([GPS, CS], F32, tag="gsq")
            nc.vector.tensor_mul(out=gsq, in0=gtmp[:, 0:CS], in1=gtmp[:, 0:CS])
            e8 = small.tile([GPS, 2 * CS], F32, tag="e8")
            nc.vector.tensor_sub(out=e8[:, 0:CS], in0=gtmp[:, CS:], in1=gsq)
            nc.scalar.activation(out=e8[:, 0:CS], in_=e8[:, 0:CS], func=AF.Sqrt,
                                 bias=eps_t[:GPS], scale=1.0)
            nc.vector.reciprocal(out=e8[:, 0:CS], in_=e8[:, 0:CS])
            nc.vector.tensor_copy(out=e8[:, CS:], in_=gtmp[:, 0:CS])
            ps_e = ps_sh.tile([P, 2 * CS], F32, tag="sbank")
            nc.tensor.matmul(ps_e, lhsT=inde, rhs=e8, start=True, stop=True)
            A_sb = small.tile([P, CS], F32, tag="A")
            nc.vector.tensor_mul(out=A_sb, in0=ps_e[:, 0:CS], in1=gammaT)
            B_sb = small.tile([P, CS], F32, tag="B")
            nc.vector.tensor_mul(out=B_sb, in0=ps_e[:, CS:], in1=A_sb)
            nc.vector.tensor_sub(out=B_sb, in0=betaT, in1=B_sb)

            # ---- phase 2: hn chunks -> K, V^T, Q (one weight at a time) ----
            def hn_chunk(src_r, ch, width):
                xc = xch.tile([P, CS, width], F32, tag="xc")
                nc.sync.dma_start(out=xc, in_=src_r[:, :, ch * width:(ch + 1) * width])
                hn = hnp.tile([P, CS, width], MMDT, tag="hn")
                for s in range(CS):
                    nc.vector.tensor_scalar(
                        hn[:, s, :], xc[:, s, :],
                        scalar1=A_sb[:, s:s + 1], scalar2=B_sb[:, s:s + 1],
                        op0=ALU.mult, op1=ALU.add,
                    )
                return hn

            wk = load_weight("k", wqkv, "wt")
            for ch in range(NCHUNKS):             # K rows [co-sub, chunk]
                hn = hn_chunk(x_r, ch, NCH)
                for cs in range(CS):
                    ps_k = ps_sh.tile([P, NCH], F32, tag="sbank")
                    for s in range(CS):
                        nc.tensor.matmul(
                            ps_k, lhsT=wk[:, s, cs * P:(cs + 1) * P],
                            rhs=hn[:, s, :],
                            start=(s == 0), stop=(s == CS - 1),
                        )
                    nc.scalar.activation(
                        out=K_sb[:, cs, ch * NCH:(ch + 1) * NCH], in_=ps_k,
                        func=AF.Identity, bias=bT["k"][:, cs:cs + 1], scale=1.0,
                    )
            wv = load_weight("v", wqkv, "wt")
            for ch in range(NCHUNKS):             # V^T rows [pixel-sub, all co]
                hn = hn_chunk(x_r, ch, NCH)
                for ns in range(NCH // P):
                    ps_v = ps_sh.tile([P, C], F32, tag="sbank")
                    for s in range(CS):
                        nc.tensor.matmul(
                            ps_v, lhsT=hn[:, s, ns * P:(ns + 1) * P],
                            rhs=wv[:, s, :],
                            start=(s == 0), stop=(s == CS - 1),
                        )
                    nc.vector.tensor_copy(
                        out=VT_sb[:, ch * (NCH // P) + ns, :], in_=ps_v
                    )
            wq = load_weight("q", wqkv, "wt")
            qw_ = min(NCH, NQ)
            for ch in range(NQ // qw_):           # Q rows (own block only)
                hn = hn_chunk(xq_r, ch, qw_)
                for cs in range(CS):
                    ps_q = ps_sh.tile([P, qw_], F32, tag="sbank")
                    for s in range(CS):
                        nc.tensor.matmul(
                            ps_q, lhsT=wq[:, s, cs * P:(cs + 1) * P],
                            rhs=hn[:, s, :],
                            start=(s == 0), stop=(s == CS - 1),
                        )
                    nc.scalar.activation(
                        out=Q_sb[:, cs, ch * qw_:(ch + 1) * qw_], in_=ps_q,
                        func=AF.Identity, bias=bT["q"][:, cs:cs + 1], scale=1.0,
                    )
            load_weight("p", None, None)

        # ---- phase 3: attention + proj + residual, per query pass ---------
        with ExitStack() as st2:
            ptp = st2.enter_context(tc.tile_pool(name="ptp", bufs=3))
            onp = st2.enter_context(tc.tile_pool(name="onp", bufs=2))
            ocq = st2.enter_context(tc.tile_pool(name="ocq", bufs=1))
            outp = st2.enter_context(tc.tile_pool(name="outp", bufs=2))
            xres = st2.enter_context(tc.tile_pool(name="xres", bufs=2))
            sm2 = st2.enter_context(tc.tile_pool(name="sm2", bufs=2))

            for qp in range(QPASSES):
                q0 = qp * QP
                o_ps = []
                for _qs in range(QS):
                    o_tile = ps_o.tile([P, C], F32, tag="o", name=f"o_{qp}_{_qs}")
                    o_ps.append(o_tile)
                acc = sm2.tile([P, QP], F32, tag="acc")
                pt_q = []

                def emit_s(kt):
                    s_ps = ps_sh.tile([P, QP], F32, tag="sbank",
                                      name=f"s_ps_{qp}_{kt}")
                    for s in range(CS):
                        nc.tensor.matmul(
                            s_ps, lhsT=K_sb[:, s, kt * P:(kt + 1) * P],
                            rhs=Q_sb[:, s, q0:q0 + QP],
                            start=(s == 0), stop=(s == CS - 1),
                        )
                    pt = ptp.tile([P, QP], MMDT, tag="pt",
                                  name=f"pt_{qp}_{kt}")
                    nc.scalar.activation(out=pt, in_=s_ps, func=AF.Exp)
                    pt_q.append((kt, pt))

                emit_s(0)
                for kt in range(KT):
                    if kt + 1 < KT:
                        emit_s(kt + 1)
                    k0, pt = pt_q.pop(0)
                    assert k0 == kt
                    pt_f = pt if MMDT is F32 else pt.bitcast(F32)
                    if kt == 0:
                        nc.vector.tensor_copy(out=acc, in_=pt_f)
                    else:
                        nc.vector.tensor_add(out=acc, in0=acc, in1=pt_f)
                    last = kt == KT - 1
                    for qs in range(QS):
                        nc.tensor.matmul(
                            o_ps[qs], lhsT=pt[:, qs * P:(qs + 1) * P],
                            rhs=VT_sb[:, kt, :],
                            start=(kt == 0), stop=last,
                        )
                sums_ps = ps_sum.tile([P, QS], F32, tag="sums")
                for qs in range(QS):
                    nc.tensor.matmul(
                        sums_ps[:, qs:qs + 1],
                        lhsT=acc[:, qs * P:(qs + 1) * P], rhs=ones_r,
                        start=True, stop=True, skip_group_check=True,
                    )
                rec4 = sm2.tile([P, QS], F32, tag="rec4")
                nc.vector.reciprocal(out=rec4, in_=sums_ps)

                oc = ocq.tile([P, CS, QP], MMDT, tag="ocq")
                for qs in range(QS):
                    on = onp.tile([P, C], F32, tag="on")
                    nc.vector.tensor_scalar_mul(on, o_ps[qs], rec4[:, qs:qs + 1])
                    for cs in range(CS):
                        t_ps = ps_sh.tile([P, P], F32, tag="sbank")
                        nc.tensor.transpose(t_ps, on[:, cs * P:(cs + 1) * P], ident)
                        nc.vector.tensor_copy(
                            out=oc[:, cs, qs * P:(qs + 1) * P], in_=t_ps
                        )
                for cs in range(CS):          # proj rows [co-sub, qpass]
                    ps_p = ps_sh.tile([P, QP], F32, tag="sbank")
                    for s in range(CS):
                        nc.tensor.matmul(
                            ps_p, lhsT=wpT[:, s, cs * P:(cs + 1) * P],
                            rhs=oc[:, s, :],
                            start=(s == 0), stop=(s == CS - 1),
                        )
                    xr_t = xres.tile([P, QP], F32, tag="xr")
                    nc.sync.dma_start(out=xr_t, in_=xq_r[:, cs, q0:q0 + QP])
                    ot = outp.tile([P, QP], F32, tag="ot")
                    nc.vector.tensor_scalar_add(ot, ps_p, bT["p"][:, cs:cs + 1])
                    nc.vector.tensor_add(out=ot, in0=ot, in1=xr_t)
                    nc.sync.dma_start(out=out_r[:, cs, q0:q0 + QP], in_=ot)

    nc.finalize()
    return nc


def make_consts(P=128, cpg=16):
    GPS = P // cpg
    indg = np.zeros((P, GPS), np.float32)
    for p in range(P):
        indg[p, p // cpg] = 1.0
    inde = indg.T.copy()
    return {
        "indg": indg,
        "inde": inde,
        "ident": np.eye(P, dtype=np.float32),
    }


def make_in_maps(x, gn_w, gn_b, q_w, q_b, k_w, k_b, v_w, v_b, proj_w, proj_b,
                 n_cores=8, G=32):
    """v1: shard full inputs into per-core single-pack input maps."""
    f = lambda a: np.ascontiguousarray(np.asarray(a, dtype=np.float32))
    x = f(x)
    b, c, h, w = x.shape
    n = h * w
    qblocks = n_cores // b
    nq = n // qblocks
    cs = c // 128
    scale = np.float32(c ** -0.5)
    xf = x.reshape(b, c, n)
    off, M = pack_offsets(c, n, nq)

    def to_pcs(v):                       # [C] -> [128, CS] (c = 128*s + p)
        return np.ascontiguousarray(np.asarray(v, np.float32).reshape(cs, 128).T)

    consts = make_consts(cpg=c // G)
    common = np.zeros((c, M), np.float32)

    def put(name, arr):
        o, wdt = off[name]
        arr = np.asarray(arr, np.float32)
        assert arr.shape[1] == wdt, (name, arr.shape, wdt)
        common[:arr.shape[0], o:o + wdt] = arr

    put("wqt", f(q_w).T * scale)
    put("wkt", f(k_w).T)
    put("wvt", f(v_w).T)
    put("wpt", f(proj_w).T)
    put("bqT", to_pcs(f(q_b) * scale))
    put("bkT", to_pcs(k_b))
    put("bpT", to_pcs(f(proj_w) @ f(v_b) + f(proj_b)))
    put("gammaT", to_pcs(gn_w))
    put("betaT", to_pcs(gn_b))
    put("indg", consts["indg"])
    put("inde", consts["inde"])
    put("ident", consts["ident"])

    in_maps = []
    for i in range(n_cores):
        bi, qi = divmod(i, qblocks)
        pk = common.copy()
        xo, _ = off["x"]
        pk[:, xo:xo + n] = xf[bi]
        qo, _ = off["xq"]
        pk[:, qo:qo + nq] = xf[bi][:, qi * nq:(qi + 1) * nq]
        in_maps.append({"pack": pk})
    return in_maps, (b, c, h, w, n, nq, qblocks)


_PROGRAM_CACHE = {}


def _get_program(C, G, N, NQ, precision="bf16"):
    key = (C, G, N, NQ, precision)
    if key not in _PROGRAM_CACHE:
        if precision == "bf16":
            _PROGRAM_CACHE[key] = build_program_v2(C=C, G=G, N=N, NQ=NQ)
        else:
            _PROGRAM_CACHE[key] = build_program(C=C, G=G, N=N, NQ=NQ,
                                                precision=precision)
    return _PROGRAM_CACHE[key]


def prepare(inputs, precision="bf16", n_cores=8):
    """Build (in_maps, meta, nc) for the given precision variant."""
    mk = make_in_maps_v2 if precision == "bf16" else make_in_maps
    in_maps, meta = mk(**inputs)
    b, c, h, w, n, nq, qblocks = meta
    nc = _get_program(C=c, G=32, N=n, NQ=nq, precision=precision)
    return in_maps, meta, nc


def kernel(x, gn_w, gn_b, q_w, q_b, k_w, k_b, v_w, v_b, proj_w, proj_b):
    from concourse.bass_utils import run_bass_kernel_spmd

    in_maps, (b, c, h, w, n, nq, qblocks), nc = prepare(dict(
        x=x, gn_w=gn_w, gn_b=gn_b, q_w=q_w, q_b=q_b, k_w=k_w, k_b=k_b,
        v_w=v_w, v_b=v_b, proj_w=proj_w, proj_b=proj_b))
    n_cores = 8
    res = run_bass_kernel_spmd(nc, in_maps, list(range(n_cores))).results
    out = np.empty((b, c, n), np.float32)
    for i in range(n_cores):
        bi, qi = divmod(i, qblocks)
        out[bi, :, qi * nq:(qi + 1) * nq] = res[i]["out"]
    return out.reshape(b, c, h, w)


# revision 46
# speedup vs baseline: 1.0420x; 1.0420x over previous
"""AttnBlock (GroupNorm -> QKV 1x1 -> full NxN attention -> proj -> residual)
for Trainium2, SPMD over 8 NeuronCores.

Sharding: data-parallel over batch (2) x query-pixel blocks (4 of 1024 px).
Each core receives its batch image x [C, N]; K and V^T are computed
redundantly per batch group, queries are disjoint.  The host ROTATES the
pixel axis per core so each core's query block is always columns [0, NQ)
of its x copy (attention is permutation-invariant over keys, GN stats are
order-free), so no separate xq input is needed and the residual is read
from resident x (bf16: ~1.5e-3 rel err vs the 2e-2 gate).  No collectives.

ALL inputs are packed into ONE bf16 DRAM tensor (fp32 sections stored as
byte pairs and bitcast at use): the axon PJRT proxy charges ~40us of
dispatch overhead PER ARGUMENT per call, so the original 17 inputs cost
~0.7ms of pure dispatch per invocation.

Per-core structure (precision="bf16", the default):
  0. DMA everything once on ONE prioritized queue (x pieces first, 4KB
     contiguous per partition); x stays RESIDENT in SBUF as bf16.
  1. GroupNorm stats split across engines so neither serializes the head:
     DVE bn_stats on 6 windows per channel-subtile, ACT raw sum/sum-sq
     (accum_out) on 2; merged, then tiny indicator matmuls do the
     cross-partition group reduce -> per-channel A, B
  2. fused single sweep over resident x chunks: hn = A*x + B (bf16) ->
     matmuls into resident K [c,n] and V^T [n,c]; Q emitted for the first
     NQ/512 chunks (attn scale folded into wq/bq on host; k-bias dropped:
     softmax over keys is invariant to a per-query additive constant)
  3. attention per 512-query pass, streamed over 32 key tiles:
     S[k,q] = K^T@Q -> exp (no max subtraction: scores ~ N(0,1)) -> pt
     bf16; AV accumulated in PSUM directly in [c,q] layout (lhsT = V^T
     tile) so no PE transposes are needed; softmax denominators: DVE
     running acc over pt tiles, then ONE all-ones matmul broadcasts
     per-query sums to all partitions; normalization is folded AFTER
     proj: out = (Wp O) * rec + (bp + Wp bv folded) + x, computed in
     half-tiles pipelined across DVE -> DMA so the final drain is short.

Matmul operands are bf16 (1 PE cycle/row, vs 4 for fp32); PSUM
accumulation is fp32.  Cost-model device time ~214us vs ~785us for the
original fp32 kernel (PE busy ~179us of that).
precision="tf32"/"fp32" build the v1 unfused single-pack variant.
"""

from contextlib import ExitStack

import numpy as np

import concourse.bacc as bacc
import concourse.bass as bass
import concourse.mybir as mybir
import concourse.tile as tile

F32 = mybir.dt.float32
F32R = mybir.dt.float32r
BF16 = mybir.dt.bfloat16
AF = mybir.ActivationFunctionType
ALU = mybir.AluOpType


# ---------------------------------------------------------------------------
# v2: bf16 fused kernel
# ---------------------------------------------------------------------------

def pack_offsets_v2(C=512, N=4096, NQ=1024):
    """Column offsets in the single packed [C, Mb] bf16 input tensor.

    fp32 sections are stored as raw bytes occupying 2x bf16 columns and
    bitcast to f32 on the DRAM AP at use (offsets stay even => 4B aligned).
    """
    P = 128
    CS = C // P
    offb, o = {}, 0
    for name, w in (("x", N),
                    ("indg", 2 * 8), ("inde", 2 * P), ("gammaT", 2 * CS),
                    ("betaT", 2 * CS), ("bqT", 2 * CS), ("bpT", 2 * CS),
                    ("wqt", C), ("wkt", C), ("wvt", C), ("wpt", C)):
        offb[name] = (o, w)
        o += w
    return offb, o


def build_program_v2(C=512, G=32, N=4096, NQ=1024, eps=1e-5):
    P = 128
    CS = C // P                  # channel subtiles
    KT = N // P                  # key tiles
    NCH = 512                    # x chunk width (= bn_stats window max)
    NCHUNKS = N // NCH
    QP = 512                     # query-pass width (one PSUM bank of fp32)
    QPASSES = NQ // QP
    cpg = C // G                 # channels per group
    GPS = P // cpg               # groups per channel-subtile
    assert C % P == 0 and N % NCH == 0 and NQ % QP == 0 and P % cpg == 0

    offb, Mb = pack_offsets_v2(C, N, NQ)

    nc = bacc.Bacc(None, target_bir_lowering=False)

    packb_d = nc.dram_tensor("packb", [C, Mb], BF16, kind="ExternalInput")
    out_d = nc.dram_tensor("out", [C, NQ], F32, kind="ExternalOutput")

    def bcol(name):
        o, w = offb[name]
        return packb_d[:, o:o + w]

    def fcol(name, rows=C):
        o, w = offb[name]
        return packb_d[0:rows, o:o + w].bitcast(F32)

    out_r = out_d[:, :].rearrange("(s p) n -> p s n", p=P)

    with tile.TileContext(nc) as tc, ExitStack() as st:
        const = st.enter_context(tc.tile_pool(name="const", bufs=1))
        big = st.enter_context(tc.tile_pool(name="big", bufs=1))
        small = st.enter_context(tc.tile_pool(name="small", bufs=1))
        ps_sh = st.enter_context(tc.tile_pool(name="ps_sh", bufs=3, space="PSUM"))
        ps_o = st.enter_context(tc.tile_pool(name="ps_o", bufs=4, space="PSUM"))
        ps_den = st.enter_context(tc.tile_pool(name="ps_den", bufs=1, space="PSUM"))

        # ---- resident tensors / constants ---------------------------------
        # x[c, n] resident as separate per-(subtile, half) tiles so the
        # phase-1 bn_stats only depend on the piece they read (whole-tile
        # deps would stall stats until the LAST x DMA).
        NH = N // 2
        X_pc = [[big.tile([P, NH], BF16, tag=f"X{s}{hh}", name=f"X_{s}_{hh}")
                 for hh in range(2)] for s in range(CS)]

        def xsl(s, lo, wdt):
            hh, off = divmod(lo, NH)
            assert off + wdt <= NH
            return X_pc[s][hh][:, off:off + wdt]
        K_sb = big.tile([P, CS, N], BF16, tag="K")        # K[co, n]
        VT_sb = big.tile([P, KT, C], BF16, tag="VT")      # V^T[n, co]
        Q_sb = big.tile([P, CS, NQ], BF16, tag="Q")       # Q[co, nq] scaled
        W_sb = {}
        for w in ("q", "k", "v", "p"):
            W_sb[w] = big.tile([P, CS, C], BF16, tag=f"w{w}", name=f"w_{w}")

        # ALL input DMAs go on the sync queue in priority order: x pieces
        # (phase-1 critical path) -> tiny consts -> weights -> xq -> xqf.
        # A single ordered queue stops the later transfers from stealing
        # HBM bandwidth from x, which gates the GroupNorm stats -> A/B ->
        # everything else dependency chain.  x is loaded per (channel
        # subtile, half): [128, 2048] bf16 pieces = 4KB contiguous per
        # partition (efficient DMA), with bn_stats chasing each piece.
        x_r = bcol("x").rearrange("(s p) n -> p s n", p=P)
        for s in range(CS):
            for hh in range(2):
                nc.sync.dma_start(
                    out=X_pc[s][hh],
                    in_=x_r[:, s, hh * NH:(hh + 1) * NH])

        indg = const.tile([P, GPS], F32, tag="indg")
        nc.sync.dma_start(out=indg, in_=fcol("indg", P)[:, 0:GPS])
        inde = const.tile([GPS, P], F32, tag="inde")
        nc.sync.dma_start(out=inde, in_=fcol("inde", GPS))
        gammaT = const.tile([P, CS], F32, tag="gammaT")
        nc.sync.dma_start(out=gammaT, in_=fcol("gammaT", P))
        betaT = const.tile([P, CS], F32, tag="betaT")
        nc.sync.dma_start(out=betaT, in_=fcol("betaT", P))
        bqT = const.tile([P, CS], F32, tag="bqT")
        nc.sync.dma_start(out=bqT, in_=fcol("bqT", P))
        bpT = const.tile([P, CS], F32, tag="bpT")
        nc.sync.dma_start(out=bpT, in_=fcol("bpT", P))

        for w in ("k", "v", "q", "p"):
            nc.sync.dma_start(out=W_sb[w], in_=bcol(f"w{w}t").rearrange(
                "(s p) c -> p s c", p=P))
        onesM_f = const.tile([P, P], F32, tag="onesM_f")
        nc.vector.memset(onesM_f, 1.0)
        onesM = const.tile([P, P], F32R, tag="onesM")
        nc.vector.tensor_copy(out=onesM, in_=onesM_f)
        eps_t = const.tile([P, 1], F32, tag="eps")
        nc.vector.memset(eps_t, eps)

        # ---- phase 1: group-norm stats over resident x --------------------
        # The 32 [128, 512] stat windows are split across two engines so
        # neither serializes the head: per channel-subtile, 6 windows go to
        # DVE bn_stats and 2 (w == 3 of each half) to ACT as raw sum +
        # sum-of-squares via accum_out.  (Pool supports neither
        # TensorScalarPtr nor free-axis reductions, so only ACT can help.)
        stats_all = small.tile([P, CS, 6, 6], F32, tag="stats")
        sxa = small.tile([P, CS, 2, 2], F32, tag="sxa")  # raw windows
        junk_a = small.tile([P, NCH], F32, tag="junk_a")
        for s in range(CS):
            for hh in range(2):
                for w in range(4):
                    ch = 4 * hh + w
                    xs = xsl(s, ch * NCH, NCH)
                    if w < 3:
                        nc.vector.bn_stats(
                            out=stats_all[:, s, 3 * hh + w, :], in_=xs)
                    else:
                        nc.scalar.activation(
                            out=junk_a, in_=xs, func=AF.Copy,
                            accum_out=sxa[:, s, hh, 0:1])
                        nc.scalar.activation(
                            out=junk_a, in_=xs, func=AF.Square,
                            accum_out=sxa[:, s, hh, 1:2])
        mv = small.tile([P, CS, 2], F32, tag="mv")
        for s in range(CS):
            nc.vector.bn_aggr(out=mv[:, s, :], in_=stats_all[:, s, :, :])

        # merge the DVE windows (mean/var over N_D px per channel) with the
        # ACT raw sums (N - N_D px) into per-channel mean / E[x^2] (rhs8),
        # group-reduced via indicator matmul below.
        N_D = 6.0 * NCH
        sxt = small.tile([P, CS, 2], F32, tag="sxt")
        nc.vector.tensor_add(out=sxt, in0=sxa[:, :, 0, :], in1=sxa[:, :, 1, :])
        rhs8 = small.tile([P, 2 * CS], F32, tag="rhs8")
        # mean_tot = (mean_d * N_D + sx_raw) / N
        nc.vector.tensor_scalar_mul(rhs8[:, 0:CS], mv[:, :, 0], N_D / float(N))
        nc.vector.tensor_scalar_mul(rhs8[:, CS:], sxt[:, :, 0], 1.0 / float(N))
        nc.vector.tensor_add(out=rhs8[:, 0:CS], in0=rhs8[:, 0:CS],
                             in1=rhs8[:, CS:])
        # E[x^2]_tot = ((var_d + mean_d^2) * N_D + sxx_raw) / N
        ex2 = small.tile([P, CS], F32, tag="ex2")
        nc.vector.tensor_mul(out=ex2, in0=mv[:, :, 0], in1=mv[:, :, 0])
        nc.vector.tensor_add(out=ex2, in0=ex2, in1=mv[:, :, 1])
        nc.vector.tensor_scalar_mul(ex2, ex2, N_D / float(N))
        nc.vector.tensor_scalar_mul(rhs8[:, CS:], sxt[:, :, 1], 1.0 / float(N))
        nc.vector.tensor_add(out=rhs8[:, CS:], in0=rhs8[:, CS:], in1=ex2)
        ps_g = ps_sh.tile([GPS, 2 * CS], F32, tag="sbank")
        nc.tensor.matmul(ps_g, lhsT=indg, rhs=rhs8, start=True, stop=True)
        gtmp = small.tile([GPS, 2 * CS], F32, tag="gtmp")
        nc.vector.tensor_scalar_mul(gtmp, ps_g, 1.0 / cpg)
        gsq = small.tile([GPS, CS], F32, tag="gsq")
        nc.vector.tensor_mul(out=gsq, in0=gtmp[:, 0:CS], in1=gtmp[:, 0:CS])
        e8 = small.tile([GPS, 2 * CS], F32, tag="e8")
        nc.vector.tensor_sub(out=e8[:, 0:CS], in0=gtmp[:, CS:], in1=gsq)
        nc.scalar.activation(out=e8[:, 0:CS], in_=e8[:, 0:CS], func=AF.Sqrt,
                             bias=eps_t[:GPS], scale=1.0)
        nc.vector.reciprocal(out=e8[:, 0:CS], in_=e8[:, 0:CS])
        nc.vector.tensor_copy(out=e8[:, CS:], in_=gtmp[:, 0:CS])
        ps_e = ps_sh.tile([P, 2 * CS], F32, tag="sbank")
        nc.tensor.matmul(ps_e, lhsT=inde, rhs=e8, start=True, stop=True)
        A_sb = small.tile([P, CS], F32, tag="A")
        nc.vector.tensor_mul(out=A_sb, in0=ps_e[:, 0:CS], in1=gammaT)
        B_sb = small.tile([P, CS], F32, tag="B")
        nc.vector.tensor_mul(out=B_sb, in0=ps_e[:, CS:], in1=A_sb)
        nc.vector.tensor_sub(out=B_sb, in0=betaT, in1=B_sb)

        # ---- phase 2: fused hn -> K, V^T sweep; then Q --------------------
        with ExitStack() as st1:
            hnp = st1.enter_context(tc.tile_pool(name="hnp", bufs=2))

            def hn_chunk(get, name):
                # one tile per channel-subtile: the first K matmul of the
                # chunk can start as soon as hn[0] lands (finer deps)
                hn = [hnp.tile([P, NCH], BF16, tag=f"hn{s}",
                               name=f"{name}_{s}") for s in range(CS)]
                for s in range(CS):
                    nc.vector.tensor_scalar(
                        hn[s], get(s),
                        scalar1=A_sb[:, s:s + 1], scalar2=B_sb[:, s:s + 1],
                        op0=ALU.mult, op1=ALU.add,
                    )
                return hn

            for ch in range(NCHUNKS):
                hn = hn_chunk(lambda s, c=ch: xsl(s, c * NCH, NCH),
                              f"hn_{ch}")
                for cs in range(CS):          # K rows [co-sub, chunk]
                    ps_k = ps_sh.tile([P, NCH], F32, tag="sbank")
                    for s in range(CS):
                        nc.tensor.matmul(
                            ps_k, lhsT=W_sb["k"][:, s, cs * P:(cs + 1) * P],
                            rhs=hn[s],
                            start=(s == 0), stop=(s == CS - 1),
                        )
                    nc.scalar.activation(
                        out=K_sb[:, cs, ch * NCH:(ch + 1) * NCH], in_=ps_k,
                        func=AF.Copy)
                for ns in range(NCH // P):    # V^T rows [pixel-sub, all co]
                    ps_v = ps_sh.tile([P, C], F32, tag="sbank")
                    for s in range(CS):
                        nc.tensor.matmul(
                            ps_v, lhsT=hn[s][:, ns * P:(ns + 1) * P],
                            rhs=W_sb["v"][:, s, :],
                            start=(s == 0), stop=(s == CS - 1),
                        )
                    nc.vector.tensor_copy(
                        out=VT_sb[:, ch * (NCH // P) + ns, :], in_=ps_v)
                if ch < NQ // NCH:
                    # this core's query block is pixels [0, NQ) (the host
                    # rotates the pixel axis per core), so Q comes from the
                    # same hn chunks as K/V
                    for cs in range(CS):
                        ps_q = ps_sh.tile([P, NCH], F32, tag="sbank")
                        for s in range(CS):
                            nc.tensor.matmul(
                                ps_q,
                                lhsT=W_sb["q"][:, s, cs * P:(cs + 1) * P],
                                rhs=hn[s],
                                start=(s == 0), stop=(s == CS - 1),
                            )
                        nc.scalar.activation(
                            out=Q_sb[:, cs, ch * NCH:(ch + 1) * NCH],
                            in_=ps_q, func=AF.Identity,
                            bias=bqT[:, cs:cs + 1], scale=1.0)

        # ---- phase 3: attention + proj + residual, per query pass ---------
        with ExitStack() as st2:
            ptp = st2.enter_context(tc.tile_pool(name="ptp", bufs=3))
            ocq = st2.enter_context(tc.tile_pool(name="ocq", bufs=1))
            outp = st2.enter_context(tc.tile_pool(name="outp", bufs=2))
            sm2 = st2.enter_context(tc.tile_pool(name="sm2", bufs=2))

            for qp in range(QPASSES):
                q0 = qp * QP
                o_ps = [ps_o.tile([P, QP], F32, tag="o", name=f"o_{qp}_{cs}")
                        for cs in range(CS)]
                # denominator accumulator (f32r so the all-ones matmul can
                # read it; DVE reads go through a f32 bitcast)
                acc = sm2.tile([P, QP], F32R, tag="acc")
                pt_q = []

                def emit_s(kt):
                    s_ps = ps_sh.tile([P, QP], F32, tag="sbank",
                                      name=f"s_ps_{qp}_{kt}")
                    for s in range(CS):
                        nc.tensor.matmul(
                            s_ps, lhsT=K_sb[:, s, kt * P:(kt + 1) * P],
                            rhs=Q_sb[:, s, q0:q0 + QP],
                            start=(s == 0), stop=(s == CS - 1),
                        )
                    pt = ptp.tile([P, QP], BF16, tag="pt",
                                  name=f"pt_{qp}_{kt}")
                    nc.scalar.activation(out=pt, in_=s_ps, func=AF.Exp)
                    pt_q.append((kt, pt))

                emit_s(0)
                for kt in range(KT):
                    if kt + 1 < KT:
                        emit_s(kt + 1)
                    k0, pt = pt_q.pop(0)
                    assert k0 == kt
                    if kt == 0:
                        nc.vector.tensor_copy(out=acc, in_=pt)
                    else:
                        nc.vector.tensor_add(out=acc, in0=acc.bitcast(F32),
                                             in1=pt)
                    last = kt == KT - 1
                    for cs in range(CS):      # O[c, q] directly
                        nc.tensor.matmul(
                            o_ps[cs], lhsT=VT_sb[:, kt, cs * P:(cs + 1) * P],
                            rhs=pt,
                            start=(kt == 0), stop=last,
                        )
                # denominators broadcast to every partition in one matmul
                den_ps = ps_den.tile([P, QP], F32, tag="den")
                nc.tensor.matmul(den_ps, lhsT=onesM, rhs=acc,
                                 start=True, stop=True)
                rec = sm2.tile([P, QP], F32, tag="rec")
                nc.vector.reciprocal(out=rec, in_=den_ps)

                oc = ocq.tile([P, CS, QP], BF16, tag="ocq")
                for cs in range(CS):
                    nc.scalar.activation(out=oc[:, cs, :], in_=o_ps[cs],
                                         func=AF.Copy)
                for cs in range(CS):          # proj rows [co-sub, qpass]
                    ps_p = ps_sh.tile([P, QP], F32, tag="sbank")
                    for s in range(CS):
                        nc.tensor.matmul(
                            ps_p, lhsT=W_sb["p"][:, s, cs * P:(cs + 1) * P],
                            rhs=oc[:, s, :],
                            start=(s == 0), stop=(s == CS - 1),
                        )
                    # epilogue in half-tiles: DVE (psum*rec) -> Pool (+bias
                    # +residual) -> DMA pipeline so the final drain is short
                    HQ = QP // 2
                    for h in range(2):
                        lo = h * HQ
                        t1 = outp.tile([P, HQ], F32, tag=f"t1{h}",
                                       name=f"t1_{qp}_{cs}_{h}")
                        nc.vector.tensor_mul(out=t1, in0=ps_p[:, lo:lo + HQ],
                                             in1=rec[:, lo:lo + HQ])
                        ot = outp.tile([P, HQ], F32, tag=f"ot{h}",
                                       name=f"ot_{qp}_{cs}_{h}")
                        nc.vector.scalar_tensor_tensor(
                            out=ot, in0=t1, scalar=bpT[:, cs:cs + 1],
                            in1=xsl(cs, q0 + lo, HQ),
                            op0=ALU.add, op1=ALU.add)
                        (nc.sync if (2 * cs + h) % 2 == 0 else
                         nc.scalar).dma_start(
                            out=out_r[:, cs, q0 + lo:q0 + lo + HQ], in_=ot)

    nc.finalize()
    return nc


def make_in_maps_v2(x, gn_w, gn_b, q_w, q_b, k_w, k_b, v_w, v_b, proj_w,
                    proj_b, n_cores=8, G=32):
    """Shard full inputs into per-core packed input maps (biases folded)."""
    NPBF = mybir.dt.np(BF16)
    f = lambda a: np.ascontiguousarray(np.asarray(a, dtype=np.float32))
    x = f(x)
    b, c, h, w = x.shape
    n = h * w
    qblocks = n_cores // b
    nq = n // qblocks
    cs = c // 128
    scale = np.float32(c ** -0.5)
    xf = x.reshape(b, c, n)
    offb, Mb = pack_offsets_v2(c, n, nq)

    def to_pcs(v):                       # [C] -> [128, CS] (c = 128*s + p)
        return np.ascontiguousarray(np.asarray(v, np.float32).reshape(cs, 128).T)

    P = 128
    cpg = c // G
    GPS = P // cpg
    indg = np.zeros((P, 8), np.float32)
    for p in range(P):
        indg[p, p // cpg] = 1.0
    inde = np.ascontiguousarray(indg[:, :GPS].T)

    commonb = np.zeros((c, Mb), NPBF)

    def putb(name, arr):
        o, wdt = offb[name]
        commonb[:, o:o + wdt] = np.asarray(arr).astype(NPBF)

    def putf(buf, name, arr):
        # embed raw fp32 bytes into the bf16 pack (2 bf16 cols per f32 col)
        o, wdt = offb[name]
        arr = np.asarray(arr, np.float32)
        rows = arr.shape[0]
        tmp = np.zeros((rows, wdt), NPBF)
        tmp.view(np.float32)[...] = arr
        buf[:rows, o:o + wdt] = tmp

    putb("wqt", f(q_w).T * scale)
    putb("wkt", f(k_w).T)
    putb("wvt", f(v_w).T)
    putb("wpt", f(proj_w).T)
    putf(commonb, "bqT", to_pcs(f(q_b) * scale))
    putf(commonb, "bpT", to_pcs(f(proj_w) @ f(v_b) + f(proj_b)))
    putf(commonb, "gammaT", to_pcs(gn_w))
    putf(commonb, "betaT", to_pcs(gn_b))
    putf(commonb, "indg", indg)
    putf(commonb, "inde", inde)

    in_maps = []
    for i in range(n_cores):
        bi, qi = divmod(i, qblocks)
        pkb = commonb.copy()
        xo, _ = offb["x"]
        # rotate the pixel axis so this core's query block sits at columns
        # [0, nq): attention is permutation-invariant over keys and the GN
        # stats are order-free, so only the query slice selection changes
        pkb[:, xo:xo + n] = np.roll(
            xf[bi], -qi * nq, axis=1).astype(NPBF)
        in_maps.append({"packb": pkb})
    return in_maps, (b, c, h, w, n, nq, qblocks)


# ---------------------------------------------------------------------------
# v1: single-pack fp32/tf32 kernel (kept for comparison; see git history of
# the docstring for the full description)
# ---------------------------------------------------------------------------

def pack_offsets(C=512, N=4096, NQ=1024):
    """Column offsets in the packed [C, M] fp32 input tensor."""
    P = 128
    CS = C // P
    off = {}
    o = 0
    for name, w in (("x", N), ("xq", NQ), ("wqt", C), ("wkt", C),
                    ("wvt", C), ("wpt", C), ("bqT", CS), ("bkT", CS),
                    ("bpT", CS), ("gammaT", CS), ("betaT", CS),
                    ("indg", P // (C // 32)), ("inde", P), ("ident", P)):
        off[name] = (o, w)
        o += w
    return off, o


def build_program(C=512, G=32, N=4096, NQ=1024, eps=1e-5, precision="tf32"):
    """Emit the per-core Bass program (SPMD; per-core data differs only)."""
    P = 128
    CS = C // P                  # channel subtiles
    KT = N // P                  # key/pixel tiles
    NCH = min(512, N)            # streamed x chunk (pixels); also bn window
    NCHUNKS = N // NCH
    QP = min(512, NQ)            # query-pass width
    QPASSES = NQ // QP
    QS = QP // P                 # query subtiles per pass
    cpg = C // G                 # channels per group
    GPS = P // cpg               # groups per channel-subtile
    assert C % P == 0 and N % P == 0 and NQ % QP == 0 and P % cpg == 0
    MMDT = F32R if precision == "tf32" else F32

    off, M = pack_offsets(C, N, NQ)

    nc = bacc.Bacc(None, target_bir_lowering=False)

    pack_d = nc.dram_tensor("pack", [C, M], F32, kind="ExternalInput")
    out_d = nc.dram_tensor("out", [C, NQ], F32, kind="ExternalOutput")

    def pcol(name):
        o, w = off[name]
        return pack_d[:, o:o + w]

    def prows(name, rows):
        o, w = off[name]
        return pack_d[0:rows, o:o + w]

    x_r = pcol("x").rearrange("(s p) n -> p s n", p=P)
    xq_r = pcol("xq").rearrange("(s p) n -> p s n", p=P)
    out_r = out_d[:, :].rearrange("(s p) n -> p s n", p=P)

    with tile.TileContext(nc) as tc, ExitStack() as st:
        const = st.enter_context(tc.tile_pool(name="const", bufs=1))
        big = st.enter_context(tc.tile_pool(name="big", bufs=1))
        small = st.enter_context(tc.tile_pool(name="small", bufs=1))
        ps_sh = st.enter_context(tc.tile_pool(name="ps_sh", bufs=3, space="PSUM"))
        ps_o = st.enter_context(tc.tile_pool(name="ps_o", bufs=QS, space="PSUM"))
        ps_sum = st.enter_context(tc.tile_pool(name="ps_sum", bufs=1, space="PSUM"))

        # ---- constants / params -------------------------------------------
        indg = const.tile([P, GPS], F32, tag="indg")
        nc.sync.dma_start(out=indg, in_=prows("indg", P))
        inde = const.tile([GPS, P], F32, tag="inde")
        nc.sync.dma_start(out=inde, in_=prows("inde", GPS))
        ident = const.tile([P, P], F32, tag="ident")
        nc.sync.dma_start(out=ident, in_=prows("ident", P))
        gammaT = const.tile([P, CS], F32, tag="gammaT")
        nc.sync.dma_start(out=gammaT, in_=prows("gammaT", P))
        betaT = const.tile([P, CS], F32, tag="betaT")
        nc.sync.dma_start(out=betaT, in_=prows("betaT", P))
        bT = {}
        for name in ("q", "k", "p"):
            t = const.tile([P, CS], F32, tag=f"bT_{name}")
            nc.sync.dma_start(out=t, in_=prows(f"b{name}T", P))
            bT[name] = t
        ones_r = const.tile([P, 1], F32, tag="ones_r")
        nc.vector.memset(ones_r, 1.0)
        eps_t = const.tile([P, 1], F32, tag="eps")
        nc.vector.memset(eps_t, eps)

        K_sb = big.tile([P, CS, N], MMDT, tag="K")       # K[co, n]
        VT_sb = big.tile([P, KT, C], MMDT, tag="VT")     # V^T[n, co]
        Q_sb = big.tile([P, CS, NQ], MMDT, tag="Q")      # Q[co, nq] (scaled)
        wpT = big.tile([P, CS, C], MMDT, tag="wpT")      # proj weight

        # ---- phase 1: group-norm stats over streamed x --------------------
        with ExitStack() as st1:
            xch = st1.enter_context(tc.tile_pool(name="xch", bufs=2))
            hnp = st1.enter_context(tc.tile_pool(name="hnp", bufs=2))
            wqkv = st1.enter_context(tc.tile_pool(name="wqkv", bufs=2))

            def load_weight(w, pool, tag):
                if pool is None:
                    t = wpT
                else:
                    t = pool.tile([P, CS, C], MMDT, tag=tag, name=f"w_{w}")
                src = pcol(f"w{w}t").rearrange("(s p) c -> p s c", p=P)
                if MMDT is F32:
                    nc.sync.dma_start(out=t, in_=src)
                else:
                    raw = xch.tile([P, CS, C], F32, tag="xc", name=f"wraw_{w}")
                    nc.sync.dma_start(out=raw, in_=src)
                    nc.vector.tensor_copy(out=t, in_=raw)  # rounds to f32r
                return t

            stats_all = small.tile([P, CS, NCHUNKS, 6], F32, tag="stats")
            dma_engs = [nc.sync, nc.scalar, nc.gpsimd]
            for ch in range(NCHUNKS):
                xc = xch.tile([P, CS, NCH], F32, tag="xc")
                dma_engs[ch % len(dma_engs)].dma_start(
                    out=xc, in_=x_r[:, :, ch * NCH:(ch + 1) * NCH])
                for s in range(CS):
                    nc.vector.bn_stats(out=stats_all[:, s, ch, :], in_=xc[:, s, :])
            mv = small.tile([P, CS, 2], F32, tag="mv")
            for s in range(CS):
                nc.vector.bn_aggr(out=mv[:, s, :], in_=stats_all[:, s, :, :])

            rhs8 = small.tile([P, 2 * CS], F32, tag="rhs8")
            nc.vector.tensor_copy(out=rhs8[:, 0:CS], in_=mv[:, :, 0])
            nc.vector.tensor_mul(out=rhs8[:, CS:], in0=mv[:, :, 0], in1=mv[:, :, 0])
            nc.vector.tensor_add(out=rhs8[:, CS:], in0=rhs8[:, CS:], in1=mv[:, :, 1])
            ps_g = ps_sh.tile([GPS, 2 * CS], F32, tag="sbank")
            nc.tensor.matmul(ps_g, lhsT=indg, rhs=rhs8, start=True, stop=True)
            gtmp = small.tile([GPS, 2 * CS], F32, tag="gtmp")
            nc.vector.tensor_scalar_mul(gtmp, ps_g, 1.0 / cpg)
            gsq = small.tile([GPS, CS], F32, tag="gsq")
            nc.vector.tensor_mul(out=gsq, in0=gtmp[:, 0:CS], in1=gtmp[:, 0:CS])
            e8 = small.tile([GPS, 2 * CS], F32, tag="e8")
            nc.vector.tensor_sub(out=e8[:, 0:CS], in0=gtmp[:, CS:], in1=gsq)
            nc.scalar.activation(out=e8[:, 0:CS], in_=e8[:, 0:CS], func=AF.Sqrt,
                                 bias=eps_t[:GPS], scale=1.0)
            nc.vector.reciprocal(out=e8[:, 0:CS], in_=e8[:, 0:CS])
            nc.vector.tensor_copy(out=e8[:, CS:], in_=gtmp[:, 0:CS])
            ps_e = ps_sh.tile([P, 2 * CS], F32, tag="sbank")
            nc.tensor.matmul(ps_e, lhsT=inde, rhs=e8, start=True, stop=True)
            A_sb = small.tile([P, CS], F32, tag="A")
            nc.vector.tensor_mul(out=A_sb, in0=ps_e[:, 0:CS], in1=gammaT)
            B_sb = small.tile([P, CS], F32, tag="B")
            nc.vector.tensor_mul(out=B_sb, in0=ps_e[:, CS:], in1=A_sb)
            nc.vector.tensor_sub(out=B_sb, in0=betaT, in1=B_sb)

            # ---- phase 2: hn chunks -> K, V^T, Q (one weight at a time) ----
            def hn_chunk(src_r, ch, width):
                xc = xch.tile([P, CS, width], F32, tag="xc")
                nc.sync.dma_start(out=xc, in_=src_r[:, :, ch * width:(ch + 1) * width])
                hn = hnp.tile([P, CS, width], MMDT, tag="hn")
                for s in range(CS):
                    nc.vector.tensor_scalar(
                        hn[:, s, :], xc[:, s, :],
                        scalar1=A_sb[:, s:s + 1], scalar2=B_sb[:, s:s + 1],
                        op0=ALU.mult, op1=ALU.add,
                    )
                return hn

            wk = load_weight("k", wqkv, "wt")
            for ch in range(NCHUNKS):             # K rows [co-sub, chunk]
                hn = hn_chunk(x_r, ch, NCH)
                for cs in range(CS):
                    ps_k = ps_sh.tile([P, NCH], F32, tag="sbank")
                    for s in range(CS):
                        nc.tensor.matmul(
                            ps_k, lhsT=wk[:, s, cs * P:(cs + 1) * P],
                            rhs=hn[:, s, :],
                            start=(s == 0), stop=(s == CS - 1),
                        )
                    nc.scalar.activation(
                        out=K_sb[:, cs, ch * NCH:(ch + 1) * NCH], in_=ps_k,
                        func=AF.Identity, bias=bT["k"][:, cs:cs + 1], scale=1.0,
                    )
            wv = load_weight("v", wqkv, "wt")
            for ch in range(NCHUNKS):             # V^T rows [pixel-sub, all co]
                hn = hn_chunk(x_r, ch, NCH)
                for ns in range(NCH // P):
                    ps_v = ps_sh.tile([P, C], F32, tag="sbank")
                    for s in range(CS):
                        nc.tensor.matmul(
                            ps_v, lhsT=hn[:, s, ns * P:(ns + 1) * P],
                            rhs=wv[:, s, :],
                            start=(s == 0), stop=(s == CS - 1),
                        )
                    nc.vector.tensor_copy(
                        out=VT_sb[:, ch * (NCH // P) + ns, :], in_=ps_v
                    )
            wq = load_weight("q", wqkv, "wt")
            qw_ = min(NCH, NQ)
            for ch in range(NQ // qw_):           # Q rows (own block only)
                hn = hn_chunk(xq_r, ch, qw_)
                for cs in range(CS):
                    ps_q = ps_sh.tile([P, qw_], F32, tag="sbank")
                    for s in range(CS):
                        nc.tensor.matmul(
                            ps_q, lhsT=wq[:, s, cs * P:(cs + 1) * P],
                            rhs=hn[:, s, :],
                            start=(s == 0), stop=(s == CS - 1),
                        )
                    nc.scalar.activation(
                        out=Q_sb[:, cs, ch * qw_:(ch + 1) * qw_], in_=ps_q,
                        func=AF.Identity, bias=bT["q"][:, cs:cs + 1], scale=1.0,
                    )
            load_weight("p", None, None)

        # ---- phase 3: attention + proj + residual, per query pass ---------
        with ExitStack() as st2:
            ptp = st2.enter_context(tc.tile_pool(name="ptp", bufs=3))
            onp = st2.enter_context(tc.tile_pool(name="onp", bufs=2))
            ocq = st2.enter_context(tc.tile_pool(name="ocq", bufs=1))
            outp = st2.enter_context(tc.tile_pool(name="outp", bufs=2))
            xres = st2.enter_context(tc.tile_pool(name="xres", bufs=2))
            sm2 = st2.enter_context(tc.tile_pool(name="sm2", bufs=2))

            for qp in range(QPASSES):
                q0 = qp * QP
                o_ps = []
                for _qs in range(QS):
                    o_tile = ps_o.tile([P, C], F32, tag="o", name=f"o_{qp}_{_qs}")
                    o_ps.append(o_tile)
                acc = sm2.tile([P, QP], F32, tag="acc")
                pt_q = []

                def emit_s(kt):
                    s_ps = ps_sh.tile([P, QP], F32, tag="sbank",
                                      name=f"s_ps_{qp}_{kt}")
                    for s in range(CS):
                        nc.tensor.matmul(
                            s_ps, lhsT=K_sb[:, s, kt * P:(kt + 1) * P],
                            rhs=Q_sb[:, s, q0:q0 + QP],
                            start=(s == 0), stop=(s == CS - 1),
                        )
                    pt = ptp.tile([P, QP], MMDT, tag="pt",
                                  name=f"pt_{qp}_{kt}")
                    nc.scalar.activation(out=pt, in_=s_ps, func=AF.Exp)
                    pt_q.append((kt, pt))

                emit_s(0)
                for kt in range(KT):
                    if kt + 1 < KT:
                        emit_s(kt + 1)
                    k0, pt = pt_q.pop(0)
                    assert k0 == kt
                    pt_f = pt if MMDT is F32 else pt.bitcast(F32)
                    if kt == 0:
                        nc.vector.tensor_copy(out=acc, in_=pt_f)
                    else:
                        nc.vector.tensor_add(out=acc, in0=acc, in1=pt_f)
                    last = kt == KT - 1
                    for qs in range(QS):
                        nc.tensor.matmul(
                            o_ps[qs], lhsT=pt[:, qs * P:(qs + 1) * P],
                            rhs=VT_sb[:, kt, :],
                            start=(kt == 0), stop=last,
                        )
                sums_ps = ps_sum.tile([P, QS], F32, tag="sums")
                for qs in range(QS):
                    nc.tensor.matmul(
                        sums_ps[:, qs:qs + 1],
                        lhsT=acc[:, qs * P:(qs + 1) * P], rhs=ones_r,
                        start=True, stop=True, skip_group_check=True,
                    )
                rec4 = sm2.tile([P, QS], F32, tag="rec4")
                nc.vector.reciprocal(out=rec4, in_=sums_ps)

                oc = ocq.tile([P, CS, QP], MMDT, tag="ocq")
                for qs in range(QS):
                    on = onp.tile([P, C], F32, tag="on")
                    nc.vector.tensor_scalar_mul(on, o_ps[qs], rec4[:, qs:qs + 1])
                    for cs in range(CS):
                        t_ps = ps_sh.tile([P, P], F32, tag="sbank")
                        nc.tensor.transpose(t_ps, on[:, cs * P:(cs + 1) * P], ident)
                        nc.vector.tensor_copy(
                            out=oc[:, cs, qs * P:(qs + 1) * P], in_=t_ps
                        )
                for cs in range(CS):          # proj rows [co-sub, qpass]
                    ps_p = ps_sh.tile([P, QP], F32, tag="sbank")
                    for s in range(CS):
                        nc.tensor.matmul(
                            ps_p, lhsT=wpT[:, s, cs * P:(cs + 1) * P],
                            rhs=oc[:, s, :],
                            start=(s == 0), stop=(s == CS - 1),
                        )
                    xr_t = xres.tile([P, QP], F32, tag="xr")
                    nc.sync.dma_start(out=xr_t, in_=xq_r[:, cs, q0:q0 + QP])
                    ot = outp.tile([P, QP], F32, tag="ot")
                    nc.vector.tensor_scalar_add(ot, ps_p, bT["p"][:, cs:cs + 1])
                    nc.vector.tensor_add(out=ot, in0=ot, in1=xr_t)
                    nc.sync.dma_start(out=out_r[:, cs, q0:q0 + QP], in_=ot)

    nc.finalize()
    return nc


def make_consts(P=128, cpg=16):
    GPS = P // cpg
    indg = np.zeros((P, GPS), np.float32)
    for p in range(P):
        indg[p, p // cpg] = 1.0
    inde = indg.T.copy()
    return {
        "indg": indg,
        "inde": inde,
        "ident": np.eye(P, dtype=np.float32),
    }


def make_in_maps(x, gn_w, gn_b, q_w, q_b, k_w, k_b, v_w, v_b, proj_w, proj_b,
                 n_cores=8, G=32):
    """v1: shard full inputs into per-core single-pack input maps."""
    f = lambda a: np.ascontiguousarray(np.asarray(a, dtype=np.float32))
    x = f(x)
    b, c, h, w = x.shape
    n = h * w
    qblocks = n_cores // b
    nq = n // qblocks
    cs = c // 128
    scale = np.float32(c ** -0.5)
    xf = x.reshape(b, c, n)
    off, M = pack_offsets(c, n, nq)

    def to_pcs(v):                       # [C] -> [128, CS] (c = 128*s + p)
        return np.ascontiguousarray(np.asarray(v, np.float32).reshape(cs, 128).T)

    consts = make_consts(cpg=c // G)
    common = np.zeros((c, M), np.float32)

    def put(name, arr):
        o, wdt = off[name]
        arr = np.asarray(arr, np.float32)
        assert arr.shape[1] == wdt, (name, arr.shape, wdt)
        common[:arr.shape[0], o:o + wdt] = arr

    put("wqt", f(q_w).T * scale)
    put("wkt", f(k_w).T)
    put("wvt", f(v_w).T)
    put("wpt", f(proj_w).T)
    put("bqT", to_pcs(f(q_b) * scale))
    put("bkT", to_pcs(k_b))
    put("bpT", to_pcs(f(proj_w) @ f(v_b) + f(proj_b)))
    put("gammaT", to_pcs(gn_w))
    put("betaT", to_pcs(gn_b))
    put("indg", consts["indg"])
    put("inde", consts["inde"])
    put("ident", consts["ident"])

    in_maps = []
    for i in range(n_cores):
        bi, qi = divmod(i, qblocks)
        pk = common.copy()
        xo, _ = off["x"]
        pk[:, xo:xo + n] = xf[bi]
        qo, _ = off["xq"]
        pk[:, qo:qo + nq] = xf[bi][:, qi * nq:(qi + 1) * nq]
        in_maps.append({"pack": pk})
    return in_maps, (b, c, h, w, n, nq, qblocks)


_PROGRAM_CACHE = {}


def _get_program(C, G, N, NQ, precision="bf16"):
    key = (C, G, N, NQ, precision)
    if key not in _PROGRAM_CACHE:
        if precision == "bf16":
            _PROGRAM_CACHE[key] = build_program_v2(C=C, G=G, N=N, NQ=NQ)
        else:
            _PROGRAM_CACHE[key] = build_program(C=C, G=G, N=N, NQ=NQ,
                                                precision=precision)
    return _PROGRAM_CACHE[key]


def prepare(inputs, precision="bf16", n_cores=8):
    """Build (in_maps, meta, nc) for the given precision variant."""
    mk = make_in_maps_v2 if precision == "bf16" else make_in_maps
    in_maps, meta = mk(**inputs)
    b, c, h, w, n, nq, qblocks = meta
    nc = _get_program(C=c, G=32, N=n, NQ=nq, precision=precision)
    return in_maps, meta, nc


def kernel(x, gn_w, gn_b, q_w, q_b, k_w, k_b, v_w, v_b, proj_w, proj_b):
    from concourse.bass_utils import run_bass_kernel_spmd

    in_maps, (b, c, h, w, n, nq, qblocks), nc = prepare(dict(
        x=x, gn_w=gn_w, gn_b=gn_b, q_w=q_w, q_b=q_b, k_w=k_w, k_b=k_b,
        v_w=v_w, v_b=v_b, proj_w=proj_w, proj_b=proj_b))
    n_cores = 8
    res = run_bass_kernel_spmd(nc, in_maps, list(range(n_cores))).results
    out = np.empty((b, c, n), np.float32)
    for i in range(n_cores):
        bi, qi = divmod(i, qblocks)
        out[bi, :, qi * nq:(qi + 1) * nq] = res[i]["out"]
    return out.reshape(b, c, h, w)
